# revision 31
# baseline (speedup 1.0000x reference)
import sys

sys.path.insert(0, "/opt/trn_rl_repo")

import numpy as np

NCORES = 8
B, FULL_N, D = 4, 2048, 1024
NH = 16
DK = 64  # head dim
HPC = NH // NCORES  # heads per core = 2
CW = HPC * DK  # output columns per core = 128
DC = D // 128  # D chunks = 8
VW = 80  # padded V width per head (64 dims + ones col + pad to %16)

# exp routing: key chunks (j % 16) in this set are exponentiated on DVE via
# the Schraudolph bit-hack (fp32 -> saturating uint8 == fp8e4m3 bit pattern);
# the rest go through ScalarE's exact Exp with fp8e4 output.
DVE_CHUNKS = frozenset({1, 4, 7, 10, 12, 15})
# byte = psum_score * (8/ln2)/8 + (56 - 24 - sigma)  [fp8e4m3, y=exp(s)/8]
A_HACK = float(1.0 / np.log(2.0))  # 1.4427 (includes the 1/sqrt(dk)=1/8 fold)
B_HACK = 31.537  # 32 - 0.463 Schraudolph mantissa-balance

_CACHE = {}
LAST_RESULTS = None


def _build(n_rows):
    """SPMD Bass program for one core. Each core computes batch-0 attention
    for its 2 heads (the reference only uses att[0]) and adds it to its
    column slice of tgt for all batches.

    All HBM traffic is fp16 (host-converted). Score matmuls use 64-contraction
    row tiling so both heads' chunks run concurrently on the PE. Softmax
    exponentials are split between ScalarE (exact Exp -> fp8e4, scaled by
    2^-3) and DVE (saturating affine-to-uint8 bit hack). P@V runs in fp8
    DoubleRow mode (256-key contraction per pass) with a ones column at
    position 64 of each 80-wide V block yielding softmax row sums for free."""
    import concourse.mybir as mybir
    import concourse.tile as tile
    from concourse import bacc
    from concourse.masks import make_identity

    fp32 = mybir.dt.float32
    fp16 = mybir.dt.float16
    fp8 = mybir.dt.float8e4
    u8 = mybir.dt.uint8

    RT = n_rows // 128  # row tiles = 16
    G = n_rows // 512  # 512-row groups = 4
    QG = G
    KC = RT  # key chunks of 128
    JP = KC // 2  # key chunk pairs = 8

    nc = bacc.Bacc(None, target_bir_lowering=False)
    tgt0t = nc.declare_dram_parameter("tgt0t", [D, n_rows], fp8, isOutput=False)
    mem0t = nc.declare_dram_parameter("mem0t", [D, n_rows], fp8, isOutput=False)
    wqt = nc.declare_dram_parameter("wqt", [D, CW], fp8, isOutput=False)
    wkt = nc.declare_dram_parameter("wkt", [D, CW], fp8, isOutput=False)
    wvt = nc.declare_dram_parameter("wvt", [D, CW], fp8, isOutput=False)
    tgtc = nc.declare_dram_parameter("tgtc", [B, n_rows, CW], fp16, isOutput=False)
    outc = nc.declare_dram_parameter("outc", [B, n_rows, CW], fp16, isOutput=True)

    Exp = mybir.ActivationFunctionType.Exp
    DR = mybir.MatmulPerfMode.DoubleRow
    mult = mybir.AluOpType.mult
    add = mybir.AluOpType.add

    with tile.TileContext(nc) as tc:
        with (
            tc.tile_pool(name="const", bufs=1) as const,
            tc.tile_pool(name="persist", bufs=1) as persist,
        ):
            ident16 = const.tile([128, 128], fp16)
            make_identity(nc, ident16)
            bias_exp = const.tile([128, 1], fp32, tag="bias")
            nc.vector.memset(bias_exp, -3.0 * float(np.log(2.0)))
            act_warm = const.tile([128, 1], fp32, tag="actw")
            nc.scalar.activation(
                out=act_warm, in_=bias_exp, func=mybir.ActivationFunctionType.Exp
            )

            KT_gs = [
                persist.tile([128, 512], fp16, tag=f"KT{g}", name=f"KT{g}")
                for g in range(G)
            ]
            QT_gs = [
                persist.tile([128, 512], fp16, tag=f"QT{g}", name=f"QT{g}")
                for g in range(G)
            ]
            # [keys, pair jp, pair half i, h*VW + dim]; ones at h*VW+64
            Vp = persist.tile([128, JP, 2, HPC * VW], fp8, tag="Vp")
            nc.gpsimd.memset(Vp, 0.0)
            Vp_h = Vp.rearrange("p jp i (h c) -> p jp i h c", h=HPC)
            nc.gpsimd.memset(Vp_h[:, :, :, :, DK : DK + 1], 1.0)

            att_sb = persist.tile([128, RT, CW], fp16, tag="att")
            tgtc_sb = persist.tile([128, B, RT, CW], fp16, tag="tgtc")

            with (
                tc.tile_pool(name="wst", bufs=1) as wst_pool,
                tc.tile_pool(name="grp", bufs=2) as grp_pool,
                tc.tile_pool(name="vtg", bufs=2) as vt_pool,
                tc.tile_pool(name="usb", bufs=2) as usb_pool,
                tc.tile_pool(name="small", bufs=8) as small_pool,
                tc.tile_pool(name="pt", bufs=2) as pt_pool,
                tc.tile_pool(name="ps_acc", bufs=1, space="PSUM") as ps_acc,
                tc.tile_pool(name="ps_w", bufs=1, space="PSUM") as ps_w,
                tc.tile_pool(name="ps_st", bufs=2, space="PSUM") as ps_st,
                tc.tile_pool(name="ps_u", bufs=2, space="PSUM") as ps_u,
            ):
                # PE warmup during the initial DMA wait (HAM un-throttle);
                # real matmuls (transpose mode doesn't count as PE-busy).
                def emit_warm(n):
                    pw = ps_acc.tile([128, 512], fp32, tag="acc")
                    for _ in range(n):
                        nc.tensor.matmul(
                            pw[:, 0:128], ident16, ident16,
                            start=True, stop=True, skip_group_check=True,
                        )

                emit_warm(64)

                WTs = {}
                for name, w in (("q", wqt), ("k", wkt), ("v", wvt)):
                    wt = wst_pool.tile([128, DC, CW], fp8, tag=f"wt{name}")
                    nc.sync.dma_start(
                        out=wt, in_=w[:, :].rearrange("(c p) q -> p c q", p=128)
                    )
                    WTs[name] = wt

                tgtT_tiles = {}

                def emit_tgt_dma(g):
                    tgtT_g = grp_pool.tile(
                        [128, DC, 512], fp8, tag="tgtTg", name=f"tgtT{g}",
                        bufs=3,
                    )
                    nc.sync.dma_start(
                        out=tgtT_g,
                        in_=tgt0t[:, g * 512 : (g + 1) * 512].rearrange(
                            "(c p) n -> p c n", p=128
                        ),
                    )
                    tgtT_tiles[g] = tgtT_g

                def emit_tgt_proj(g):
                    tgtT_g = tgtT_tiles.pop(g)
                    pq = ps_acc.tile([128, 512], fp32, tag="acc")
                    for dp in range(DC // 2):
                        nc.tensor.matmul(
                            pq,
                            WTs["q"][:, 2 * dp : 2 * dp + 2, :],
                            tgtT_g[:, 2 * dp : 2 * dp + 2, :],
                            start=(dp == 0), stop=(dp == DC // 2 - 1),
                            perf_mode=DR,
                        )
                    nc.vector.tensor_copy(out=QT_gs[g], in_=pq)

                def emit_mem_group(g):
                    memT_g = grp_pool.tile(
                        [128, DC, 512], fp8, tag="memTg", name=f"memT{g}"
                    )
                    # mem group 0 is issued from ACT (idle until the first
                    # scores exist); later groups from SP so they don't block
                    # exp calls in ACT's FIFO.
                    eng = nc.scalar if g == 0 else nc.sync
                    eng.dma_start(
                        out=memT_g,
                        in_=mem0t[:, g * 512 : (g + 1) * 512].rearrange(
                            "(c p) n -> p c n", p=128
                        ),
                    )
                    pk = ps_acc.tile([128, 512], fp32, tag="acc")
                    for dp in range(DC // 2):
                        nc.tensor.matmul(
                            pk,
                            WTs["k"][:, 2 * dp : 2 * dp + 2, :],
                            memT_g[:, 2 * dp : 2 * dp + 2, :],
                            start=(dp == 0), stop=(dp == DC // 2 - 1),
                            perf_mode=DR,
                        )
                    nc.vector.tensor_copy(out=KT_gs[g], in_=pk)
                    pv = ps_acc.tile([128, 512], fp32, tag="acc")
                    for dp in range(DC // 2):
                        nc.tensor.matmul(
                            pv,
                            WTs["v"][:, 2 * dp : 2 * dp + 2, :],
                            memT_g[:, 2 * dp : 2 * dp + 2, :],
                            start=(dp == 0), stop=(dp == DC // 2 - 1),
                            perf_mode=DR,
                        )
                    vt_g = vt_pool.tile([128, 512], fp16, tag="vtg")
                    nc.vector.tensor_copy(out=vt_g, in_=pv)
                    for t in range(4):
                        j = 4 * g + t
                        jp, i = j // 2, j % 2
                        tw = ps_w.tile([128, 128], fp16, tag="tw")
                        nc.tensor.transpose(
                            tw, vt_g[:, t * 128 : (t + 1) * 128], ident16
                        )
                        nc.vector.tensor_copy(
                            out=Vp_h[:, jp, i, :, 0:DK],
                            in_=tw.rearrange("p (h c) -> p h c", h=HPC),
                        )

                def emit_score_chunk(qg, j, pts, warm=False):
                    g, t = j // 4, j % 4
                    st = ps_st.tile(
                        [128, HPC, 512], fp32, tag="st", name=f"st{qg}_{j}"
                    )
                    for h in range(HPC):
                        hs = h * DK
                        nc.tensor.matmul(
                            st[:, h, :],
                            KT_gs[g][hs : hs + DK, t * 128 : (t + 1) * 128],
                            QT_gs[qg][hs : hs + DK, :],
                            start=True, stop=True,
                        )
                    if warm:
                        emit_warm(1)
                    jp, i = j // 2, j % 2
                    dst = pts[:, jp, i, :, :]  # [128, h, 512] fp8
                    if j in DVE_CHUNKS:
                        nc.vector.tensor_scalar(
                            out=dst.bitcast(u8), in0=st,
                            scalar1=A_HACK, scalar2=B_HACK,
                            op0=mult, op1=add,
                        )
                    else:
                        nc.scalar.activation(
                            out=dst, in_=st, func=Exp, scale=0.125, bias=bias_exp
                        )

                def emit_pv_pair(pus, jp, pts):
                    # one DoubleRow accumulation step per head; fires as soon
                    # as exp of key chunks (2jp, 2jp+1) lands in pts
                    for h in range(HPC):
                        nc.tensor.matmul(
                            pus[h],
                            Vp[:, jp, :, h * VW : (h + 1) * VW],
                            pts[:, jp, :, h, :],
                            start=(jp == 0), stop=(jp == JP - 1),
                            perf_mode=DR,
                        )

                def emit_finalize(qg, h, pu, on_act=False):
                    # on_act: route the copy + unscale through ScalarE (idle
                    # in the tail) so the two heads finalize in parallel
                    pu_sb = usb_pool.tile([VW, 512], fp16, tag="usb")
                    if on_act:
                        nc.scalar.copy(out=pu_sb, in_=pu)
                    else:
                        nc.vector.tensor_copy(out=pu_sb, in_=pu)
                    hs = h * DK
                    for s in range(4):
                        ta = ps_w.tile([128, 128], fp16, tag="tw")
                        nc.tensor.transpose(
                            ta[:, 0:VW],
                            pu_sb[:, s * 128 : (s + 1) * 128],
                            ident16[0:VW, 0:VW],
                        )
                        rec = small_pool.tile([128, 1], fp32, tag="rec")
                        nc.vector.reciprocal(rec, ta[:, DK : DK + 1])
                        if on_act:
                            nc.scalar.activation(
                                out=att_sb[:, qg * 4 + s, hs : hs + DK],
                                in_=ta[:, 0:DK],
                                func=mybir.ActivationFunctionType.Copy,
                                scale=rec,
                            )
                        else:
                            nc.vector.tensor_scalar_mul(
                                att_sb[:, qg * 4 + s, hs : hs + DK],
                                in0=ta[:, 0:DK],
                                scalar1=rec,
                            )

                # ---- Phase A: loads, projections, qg0 scores ----
                emit_tgt_dma(0)
                emit_tgt_proj(0)
                pts_tiles = {}
                pus = {}
                pts_tiles[0] = pt_pool.tile(
                    [128, JP, 2, HPC, 512], fp8, tag="pts", name="pts0"
                )
                for g in range(G):
                    emit_mem_group(g)
                    for j in range(4 * g, 4 * g + 4):
                        emit_score_chunk(0, j, pts_tiles[0])
                # tgt groups 1-3 DMA after the mem DMAs (SP chain feeds the
                # qg0 stream first); only group 1's projection runs here --
                # groups 2/3 project inside the exp-paced iterations
                for g in range(1, G):
                    emit_tgt_dma(g)
                emit_tgt_proj(1)
                for b in range(B):
                    nc.sync.dma_start(
                        out=tgtc_sb[:, b, :, :],
                        in_=tgtc[b, :, :].rearrange("(t p) c -> p t c", p=128),
                    )
                # PV of qg0, stall-paced on its own exp stream (nothing
                # PE-critical sits behind these in program order)
                pus[0] = [
                    ps_u.tile([VW, 512], fp32, tag="u", name=f"u0_{h}")
                    for h in range(HPC)
                ]
                for jp in range(JP):
                    emit_pv_pair(pus[0], jp, pts_tiles[0])

                # ---- Phase B: iteration k emits scores(k) (st-rotation paces
                # them to the exp stream), finalizes k-1, then PV(k) rides its
                # own exp heartbeat at the tail ----
                for k in range(1, QG + 1):
                    qg = k - 1  # the q-group finalized this iteration
                    if k + 1 < QG:
                        emit_tgt_proj(k + 1)
                    if k < QG:
                        pts_tiles[k] = pt_pool.tile(
                            [128, JP, 2, HPC, 512], fp8, tag="pts",
                            name=f"pts{k}",
                        )
                        for j in range(KC):
                            emit_score_chunk(k, j, pts_tiles[k])
                    for h in range(HPC):
                        emit_finalize(qg, h, pus[qg][h], on_act=(k == QG and h == 1))
                    if k < QG:
                        pus[k] = [
                            ps_u.tile([VW, 512], fp32, tag="u", name=f"u{k}_{h}")
                            for h in range(HPC)
                        ]
                        for jp in range(JP):
                            emit_pv_pair(pus[k], jp, pts_tiles[k])
                    qsl = slice(qg * 512, (qg + 1) * 512)
                    for b in range(B):
                        eng = nc.vector if k == QG else nc.gpsimd
                        eng.tensor_add(
                            out=tgtc_sb[:, b, qg * 4 : (qg + 1) * 4, :],
                            in0=tgtc_sb[:, b, qg * 4 : (qg + 1) * 4, :],
                            in1=att_sb[:, qg * 4 : (qg + 1) * 4, :],
                        )
                    for b in range(B):
                        nc.sync.dma_start(
                            out=outc[b, qsl, :].rearrange(
                                "(t p) c -> p t c", p=128
                            ),
                            in_=tgtc_sb[:, b, qg * 4 : (qg + 1) * 4, :],
                        )

    nc.finalize()
    return nc


def _get_nc(n_rows):
    if n_rows not in _CACHE:
        _CACHE[n_rows] = _build(n_rows)
    return _CACHE[n_rows]


def _run(tgt, memory, Wq, Wk, Wv, trace=False):
    global LAST_RESULTS
    from concourse.bass_utils import run_bass_kernel_spmd

    n_rows = tgt.shape[1]
    nc = _get_nc(n_rows)

    tgt = np.ascontiguousarray(tgt, dtype=np.float32)
    memory = np.ascontiguousarray(memory, dtype=np.float32)
    import ml_dtypes

    f8 = ml_dtypes.float8_e4m3
    tgt0t = np.ascontiguousarray(tgt[0].T).astype(f8)
    mem0t = np.ascontiguousarray(memory[0].T).astype(f8)

    in_maps = []
    for c in range(NCORES):
        sl = slice(c * CW, (c + 1) * CW)
        in_maps.append(
            {
                "tgt0t": tgt0t,
                "mem0t": mem0t,
                "wqt": np.ascontiguousarray(Wq[sl, :].T).astype(f8),
                "wkt": np.ascontiguousarray(Wk[sl, :].T).astype(f8),
                "wvt": np.ascontiguousarray(Wv[sl, :].T).astype(f8),
                "tgtc": np.ascontiguousarray(tgt[:, :, sl]).astype(np.float16),
            }
        )
    res = run_bass_kernel_spmd(nc, in_maps, list(range(NCORES)), trace=trace)
    LAST_RESULTS = res
    out = np.concatenate(
        [res.results[c]["outc"].astype(np.float32) for c in range(NCORES)], axis=2
    )
    return out


def kernel(tgt, memory, Wq, Wk, Wv):
    return _run(tgt, memory, Wq, Wk, Wv)


# revision 32
# speedup vs baseline: 1.0198x; 1.0198x over previous
import sys

sys.path.insert(0, "/opt/trn_rl_repo")

import numpy as np

NCORES = 8
B, FULL_N, D = 4, 2048, 1024
NH = 16
DK = 64  # head dim
HPC = NH // NCORES  # heads per core = 2
CW = HPC * DK  # output columns per core = 128
DC = D // 128  # D chunks = 8
VW = 80  # padded V width per head (64 dims + ones col + pad to %16)

# exp routing: key chunks (j % 16) in this set are exponentiated on DVE via
# the Schraudolph bit-hack (fp32 -> saturating uint8 == fp8e4m3 bit pattern);
# the rest go through ScalarE's exact Exp with fp8e4 output.
DVE_CHUNKS = frozenset({2, 5, 8, 11, 14})
# byte = psum_score * (8/ln2)/8 + (56 - 24 - sigma)  [fp8e4m3, y=exp(s)/8]
A_HACK = float(1.0 / np.log(2.0))  # 1.4427 (includes the 1/sqrt(dk)=1/8 fold)
B_HACK = 31.537  # 32 - 0.463 Schraudolph mantissa-balance

_CACHE = {}
LAST_RESULTS = None


def _build(n_rows):
    """SPMD Bass program for one core. Each core computes batch-0 attention
    for its 2 heads (the reference only uses att[0]) and adds it to its
    column slice of tgt for all batches.

    All HBM traffic is fp16 (host-converted). Score matmuls use 64-contraction
    row tiling so both heads' chunks run concurrently on the PE. Softmax
    exponentials are split between ScalarE (exact Exp -> fp8e4, scaled by
    2^-3) and DVE (saturating affine-to-uint8 bit hack). P@V runs in fp8
    DoubleRow mode (256-key contraction per pass) with a ones column at
    position 64 of each 80-wide V block yielding softmax row sums for free."""
    import concourse.mybir as mybir
    import concourse.tile as tile
    from concourse import bacc
    from concourse.masks import make_identity

    fp32 = mybir.dt.float32
    fp16 = mybir.dt.float16
    fp8 = mybir.dt.float8e4
    u8 = mybir.dt.uint8

    RT = n_rows // 128  # row tiles = 16
    G = n_rows // 512  # 512-row groups = 4
    QG = G
    KC = RT  # key chunks of 128
    JP = KC // 2  # key chunk pairs = 8

    nc = bacc.Bacc(None, target_bir_lowering=False)
    tgt0t = nc.declare_dram_parameter("tgt0t", [D, n_rows], fp8, isOutput=False)
    mem0t = nc.declare_dram_parameter("mem0t", [D, n_rows], fp8, isOutput=False)
    wqt = nc.declare_dram_parameter("wqt", [D, CW], fp8, isOutput=False)
    wkt = nc.declare_dram_parameter("wkt", [D, CW], fp8, isOutput=False)
    wvt = nc.declare_dram_parameter("wvt", [D, CW], fp8, isOutput=False)
    tgtc = nc.declare_dram_parameter("tgtc", [B, n_rows, CW], fp16, isOutput=False)
    outc = nc.declare_dram_parameter("outc", [B, n_rows, CW], fp16, isOutput=True)

    Exp = mybir.ActivationFunctionType.Exp
    DR = mybir.MatmulPerfMode.DoubleRow
    mult = mybir.AluOpType.mult
    add = mybir.AluOpType.add

    with tile.TileContext(nc) as tc:
        with (
            tc.tile_pool(name="const", bufs=1) as const,
            tc.tile_pool(name="persist", bufs=1) as persist,
        ):
            ident16 = const.tile([128, 128], fp16)
            make_identity(nc, ident16)
            bias_exp = const.tile([128, 1], fp32, tag="bias")
            nc.vector.memset(bias_exp, -3.0 * float(np.log(2.0)))
            act_warm = const.tile([128, 1], fp32, tag="actw")
            nc.scalar.activation(
                out=act_warm, in_=bias_exp, func=mybir.ActivationFunctionType.Exp
            )

            KT_gs = [
                persist.tile([128, 512], fp16, tag=f"KT{g}", name=f"KT{g}")
                for g in range(G)
            ]
            QT_gs = [
                persist.tile([128, 512], fp16, tag=f"QT{g}", name=f"QT{g}")
                for g in range(G)
            ]
            # [keys, pair jp, pair half i, h*VW + dim]; ones at h*VW+64
            Vp = persist.tile([128, JP, 2, HPC * VW], fp8, tag="Vp")
            nc.gpsimd.memset(Vp, 0.0)
            Vp_h = Vp.rearrange("p jp i (h c) -> p jp i h c", h=HPC)
            nc.gpsimd.memset(Vp_h[:, :, :, :, DK : DK + 1], 1.0)

            att_sb = persist.tile([128, RT, CW], fp16, tag="att")
            tgtc_sb = persist.tile([128, B, RT, CW], fp16, tag="tgtc")

            with (
                tc.tile_pool(name="wst", bufs=1) as wst_pool,
                tc.tile_pool(name="grp", bufs=2) as grp_pool,
                tc.tile_pool(name="vtg", bufs=2) as vt_pool,
                tc.tile_pool(name="usb", bufs=2) as usb_pool,
                tc.tile_pool(name="small", bufs=8) as small_pool,
                tc.tile_pool(name="pt", bufs=2) as pt_pool,
                tc.tile_pool(name="ps_acc", bufs=1, space="PSUM") as ps_acc,
                tc.tile_pool(name="ps_w", bufs=1, space="PSUM") as ps_w,
                tc.tile_pool(name="ps_st", bufs=2, space="PSUM") as ps_st,
                tc.tile_pool(name="ps_u", bufs=2, space="PSUM") as ps_u,
            ):
                # PE warmup during the initial DMA wait (HAM un-throttle);
                # real matmuls (transpose mode doesn't count as PE-busy).
                def emit_warm(n):
                    pw = ps_acc.tile([128, 512], fp32, tag="acc")
                    for _ in range(n):
                        nc.tensor.matmul(
                            pw[:, 0:128], ident16, ident16,
                            start=True, stop=True, skip_group_check=True,
                        )

                emit_warm(64)

                WTs = {}
                for name, w in (("q", wqt), ("k", wkt), ("v", wvt)):
                    wt = wst_pool.tile([128, DC, CW], fp8, tag=f"wt{name}")
                    nc.sync.dma_start(
                        out=wt, in_=w[:, :].rearrange("(c p) q -> p c q", p=128)
                    )
                    WTs[name] = wt

                tgtT_tiles = {}

                def emit_tgt_dma(g):
                    tgtT_g = grp_pool.tile(
                        [128, DC, 512], fp8, tag="tgtTg", name=f"tgtT{g}",
                        bufs=3,
                    )
                    nc.sync.dma_start(
                        out=tgtT_g,
                        in_=tgt0t[:, g * 512 : (g + 1) * 512].rearrange(
                            "(c p) n -> p c n", p=128
                        ),
                    )
                    tgtT_tiles[g] = tgtT_g

                def emit_tgt_proj(g):
                    tgtT_g = tgtT_tiles.pop(g)
                    pq = ps_acc.tile([128, 512], fp32, tag="acc")
                    for dp in range(DC // 2):
                        nc.tensor.matmul(
                            pq,
                            WTs["q"][:, 2 * dp : 2 * dp + 2, :],
                            tgtT_g[:, 2 * dp : 2 * dp + 2, :],
                            start=(dp == 0), stop=(dp == DC // 2 - 1),
                            perf_mode=DR,
                        )
                    nc.vector.tensor_copy(out=QT_gs[g], in_=pq)

                def emit_mem_group(g):
                    memT_g = grp_pool.tile(
                        [128, DC, 512], fp8, tag="memTg", name=f"memT{g}"
                    )
                    # mem group 0 is issued from ACT (idle until the first
                    # scores exist); later groups from SP so they don't block
                    # exp calls in ACT's FIFO.
                    eng = nc.scalar if g == 0 else nc.sync
                    eng.dma_start(
                        out=memT_g,
                        in_=mem0t[:, g * 512 : (g + 1) * 512].rearrange(
                            "(c p) n -> p c n", p=128
                        ),
                    )
                    pk = ps_acc.tile([128, 512], fp32, tag="acc")
                    for dp in range(DC // 2):
                        nc.tensor.matmul(
                            pk,
                            WTs["k"][:, 2 * dp : 2 * dp + 2, :],
                            memT_g[:, 2 * dp : 2 * dp + 2, :],
                            start=(dp == 0), stop=(dp == DC // 2 - 1),
                            perf_mode=DR,
                        )
                    nc.vector.tensor_copy(out=KT_gs[g], in_=pk)
                    pv = ps_acc.tile([128, 512], fp32, tag="acc")
                    for dp in range(DC // 2):
                        nc.tensor.matmul(
                            pv,
                            WTs["v"][:, 2 * dp : 2 * dp + 2, :],
                            memT_g[:, 2 * dp : 2 * dp + 2, :],
                            start=(dp == 0), stop=(dp == DC // 2 - 1),
                            perf_mode=DR,
                        )
                    vt_g = vt_pool.tile([128, 512], fp16, tag="vtg")
                    nc.vector.tensor_copy(out=vt_g, in_=pv)
                    for t in range(4):
                        j = 4 * g + t
                        jp, i = j // 2, j % 2
                        tw = ps_w.tile([128, 128], fp16, tag="tw")
                        nc.tensor.transpose(
                            tw, vt_g[:, t * 128 : (t + 1) * 128], ident16
                        )
                        nc.vector.tensor_copy(
                            out=Vp_h[:, jp, i, :, 0:DK],
                            in_=tw.rearrange("p (h c) -> p h c", h=HPC),
                        )

                def emit_score_chunk(qg, j, pts, warm=False):
                    g, t = j // 4, j % 4
                    st = ps_st.tile(
                        [128, HPC, 512], fp32, tag="st", name=f"st{qg}_{j}"
                    )
                    for h in range(HPC):
                        hs = h * DK
                        nc.tensor.matmul(
                            st[:, h, :],
                            KT_gs[g][hs : hs + DK, t * 128 : (t + 1) * 128],
                            QT_gs[qg][hs : hs + DK, :],
                            start=True, stop=True,
                        )
                    if warm:
                        emit_warm(1)
                    jp, i = j // 2, j % 2
                    dst = pts[:, jp, i, :, :]  # [128, h, 512] fp8
                    if j in DVE_CHUNKS:
                        nc.vector.tensor_scalar(
                            out=dst.bitcast(u8), in0=st,
                            scalar1=A_HACK, scalar2=B_HACK,
                            op0=mult, op1=add,
                        )
                    else:
                        nc.scalar.activation(
                            out=dst, in_=st, func=Exp, scale=0.125, bias=bias_exp
                        )

                def emit_pv_pair(pus, jp, pts):
                    # one DoubleRow accumulation step per head; fires as soon
                    # as exp of key chunks (2jp, 2jp+1) lands in pts
                    for h in range(HPC):
                        nc.tensor.matmul(
                            pus[h],
                            Vp[:, jp, :, h * VW : (h + 1) * VW],
                            pts[:, jp, :, h, :],
                            start=(jp == 0), stop=(jp == JP - 1),
                            perf_mode=DR,
                        )

                def emit_finalize(qg, h, pu, on_act=False):
                    # on_act: route the copy + unscale through ScalarE (idle
                    # in the tail) so the two heads finalize in parallel
                    pu_sb = usb_pool.tile([VW, 512], fp16, tag="usb")
                    if on_act:
                        nc.scalar.copy(out=pu_sb, in_=pu)
                    else:
                        nc.vector.tensor_copy(out=pu_sb, in_=pu)
                    hs = h * DK
                    for s in range(4):
                        ta = ps_w.tile([128, 128], fp16, tag="tw")
                        nc.tensor.transpose(
                            ta[:, 0:VW],
                            pu_sb[:, s * 128 : (s + 1) * 128],
                            ident16[0:VW, 0:VW],
                        )
                        rec = small_pool.tile([128, 1], fp32, tag="rec")
                        nc.vector.reciprocal(rec, ta[:, DK : DK + 1])
                        if on_act:
                            nc.scalar.activation(
                                out=att_sb[:, qg * 4 + s, hs : hs + DK],
                                in_=ta[:, 0:DK],
                                func=mybir.ActivationFunctionType.Copy,
                                scale=rec,
                            )
                        else:
                            nc.vector.tensor_scalar_mul(
                                att_sb[:, qg * 4 + s, hs : hs + DK],
                                in0=ta[:, 0:DK],
                                scalar1=rec,
                            )

                # ---- Phase A: loads, projections, qg0 scores ----
                emit_tgt_dma(0)
                emit_tgt_proj(0)
                pts_tiles = {}
                pus = {}
                pts_tiles[0] = pt_pool.tile(
                    [128, JP, 2, HPC, 512], fp8, tag="pts", name="pts0"
                )
                for g in range(G):
                    emit_mem_group(g)
                    for j in range(4 * g, 4 * g + 4):
                        emit_score_chunk(0, j, pts_tiles[0])
                # tgt groups 1-3 DMA after the mem DMAs (SP chain feeds the
                # qg0 stream first); only group 1's projection runs here --
                # groups 2/3 project inside the exp-paced iterations
                for g in range(1, G):
                    emit_tgt_dma(g)
                emit_tgt_proj(1)
                for b in range(B):
                    nc.sync.dma_start(
                        out=tgtc_sb[:, b, :, :],
                        in_=tgtc[b, :, :].rearrange("(t p) c -> p t c", p=128),
                    )
                # PV of qg0, stall-paced on its own exp stream (nothing
                # PE-critical sits behind these in program order)
                pus[0] = [
                    ps_u.tile([VW, 512], fp32, tag="u", name=f"u0_{h}")
                    for h in range(HPC)
                ]
                for jp in range(JP):
                    emit_pv_pair(pus[0], jp, pts_tiles[0])

                # ---- Phase B: iteration k emits scores(k) (st-rotation paces
                # them to the exp stream), finalizes k-1, then PV(k) rides its
                # own exp heartbeat at the tail ----
                for k in range(1, QG + 1):
                    qg = k - 1  # the q-group finalized this iteration
                    if k + 1 < QG:
                        emit_tgt_proj(k + 1)
                    if k < QG:
                        pts_tiles[k] = pt_pool.tile(
                            [128, JP, 2, HPC, 512], fp8, tag="pts",
                            name=f"pts{k}",
                        )
                        for j in range(KC):
                            emit_score_chunk(k, j, pts_tiles[k])
                    for h in range(HPC):
                        emit_finalize(qg, h, pus[qg][h], on_act=(h == 1))
                    if k < QG:
                        pus[k] = [
                            ps_u.tile([VW, 512], fp32, tag="u", name=f"u{k}_{h}")
                            for h in range(HPC)
                        ]
                        for jp in range(JP):
                            emit_pv_pair(pus[k], jp, pts_tiles[k])
                    qsl = slice(qg * 512, (qg + 1) * 512)
                    for b in range(B):
                        eng = nc.vector if k == QG else nc.gpsimd
                        eng.tensor_add(
                            out=tgtc_sb[:, b, qg * 4 : (qg + 1) * 4, :],
                            in0=tgtc_sb[:, b, qg * 4 : (qg + 1) * 4, :],
                            in1=att_sb[:, qg * 4 : (qg + 1) * 4, :],
                        )
                    for b in range(B):
                        nc.sync.dma_start(
                            out=outc[b, qsl, :].rearrange(
                                "(t p) c -> p t c", p=128
                            ),
                            in_=tgtc_sb[:, b, qg * 4 : (qg + 1) * 4, :],
                        )

    nc.finalize()
    return nc


def _get_nc(n_rows):
    if n_rows not in _CACHE:
        _CACHE[n_rows] = _build(n_rows)
    return _CACHE[n_rows]


def _run(tgt, memory, Wq, Wk, Wv, trace=False):
    global LAST_RESULTS
    from concourse.bass_utils import run_bass_kernel_spmd

    n_rows = tgt.shape[1]
    nc = _get_nc(n_rows)

    tgt = np.ascontiguousarray(tgt, dtype=np.float32)
    memory = np.ascontiguousarray(memory, dtype=np.float32)
    import ml_dtypes

    f8 = ml_dtypes.float8_e4m3
    tgt0t = np.ascontiguousarray(tgt[0].T).astype(f8)
    mem0t = np.ascontiguousarray(memory[0].T).astype(f8)

    in_maps = []
    for c in range(NCORES):
        sl = slice(c * CW, (c + 1) * CW)
        in_maps.append(
            {
                "tgt0t": tgt0t,
                "mem0t": mem0t,
                "wqt": np.ascontiguousarray(Wq[sl, :].T).astype(f8),
                "wkt": np.ascontiguousarray(Wk[sl, :].T).astype(f8),
                "wvt": np.ascontiguousarray(Wv[sl, :].T).astype(f8),
                "tgtc": np.ascontiguousarray(tgt[:, :, sl]).astype(np.float16),
            }
        )
    res = run_bass_kernel_spmd(nc, in_maps, list(range(NCORES)), trace=trace)
    LAST_RESULTS = res
    out = np.concatenate(
        [res.results[c]["outc"].astype(np.float32) for c in range(NCORES)], axis=2
    )
    return out


def kernel(tgt, memory, Wq, Wk, Wv):
    return _run(tgt, memory, Wq, Wk, Wv)
